# revision 17
# baseline (speedup 1.0000x reference)
"""Trainium2 Bass kernel for nn_Block_70944269795510 (involution block).

8 NeuronCores, data-parallel over batch (2 samples/core). Per sample:
  conv1 (PE bf16, bn1 folded into weights, bias via ACT) -> tanh -> padded-Y
      bf16 ypa [128, 2(m), NPAD] + shifted parity copy ypb
  red (PE bf16, bn folded) -> relu (ACT) -> rt bf16 [65, HW]; row 64 is a
      ones-row so the span matmul adds the span bias (and bn2 scale, folded
      into the weights) itself
  involution in 7 pixel-chunks of 448 (8 rows), both m halves per tap:
      span matmul bf16, contraction 65 -> PSUM [128, 2x448]
      product wd*ysh split across engines by tap type:
        S: DVE tensor_mul straight from PSUM (1x rate, one op, both m)
        A: ACT evac -> bf16 -> 2x DVE tensor_mul per m (3D views hit the
           DVE double-pump path; 4D views measured at 1x on hw)
        P: ACT evac -> bf16 -> GpSimd tensor_mul (otherwise-idle engine)
      accumulate: mostly PE identity-matmuls into acc-PSUM; a few taps go
      through an SBUF bf16 chain on DVE/Pool (merged into acc at the end)
      to relieve the PE instruction-issue budget
  tanh/bn2 (ACT) -> y2 bf16 -> conv3 (PE bf16) with the +x skip folded in
      as an identity matmul of xb into the same PSUM group -> bn3 evac
      (ACT) -> DMA out

Adds are flushed two tap-batches behind the spans so their products are
always ready (the PE queue is in-order; a stalled add would block later
spans). Conv phases of neighbouring samples run as filler units between
tap batches so no engine drains at sample seams.
"""

import sys

for _p in ("/opt/trn_rl_repo", "/root/.axon_site/_ro/trn_rl_repo"):
    if _p not in sys.path:
        sys.path.append(_p)

import numpy as np
import ml_dtypes
from contextlib import ExitStack

import concourse.bass as bass
import concourse.mybir as mybir
from concourse import bacc
from concourse.tile import TileContext
from concourse.bass_utils import run_bass_kernel_spmd

F32 = mybir.dt.float32
BF16 = mybir.dt.bfloat16
AF = mybir.ActivationFunctionType
ALU = mybir.AluOpType

B, C, H, W = 16, 256, 56, 56
HW = H * W
KK = 7
G, GC, RED = 16, 16, 64
EPS = 1e-5
PAD = 3
N_CORES = 8
S_PER_CORE = B // N_CORES
PW = 64
HP = H + 2 * PAD
NPAD = HP * PW

T448 = 448
CH = 448            # involution pixel-chunk (8 rows of 56)
NCH = 7             # chunks per image
CROWS = 8
KR = 65             # span contraction: 64 r-rows + ones row (bias)

# tap type by p: 'S' DVE-from-psum, 'A' ACT evac + DVE mul, 'P' ACT + Pool
def _mk_types():
    t = []
    ns = na = 0
    for i in range(49):
        if i % 4 == 1:
            t.append('P')
        elif ns * 24 <= na * 13:
            t.append('S')
            ns += 1
        else:
            t.append('A')
            na += 1
    return t

TAP_TYPES = _mk_types()
# taps accumulated on an SBUF bf16 chain instead of PE ident-adds; the
# chain runs on DVE except CHAIN_POOL taps which add on GpSimd
CHAIN_TAPS = (3, 11, 19, 23, 31, 39, 43, 47)
CHAIN_POOL = frozenset({19, 31, 43})


def _build_nc():
    nc = bacc.Bacc("TRN2", target_bir_lowering=False, debug=False)

    xbd = nc.dram_tensor("xb", [S_PER_CORE, 128, 2, HW], BF16, kind="ExternalInput").ap()
    w1d = nc.dram_tensor("w1t", [128, 2, 256], BF16, kind="ExternalInput").ap()
    rwd = nc.dram_tensor("rwt", [128, 2, 64], BF16, kind="ExternalInput").ap()
    srd = nc.dram_tensor("scred", [64, 1], F32, kind="ExternalInput").ap()
    spd = nc.dram_tensor("srep", [KR, 2 * 49 * 128], BF16, kind="ExternalInput").ap()
    w3d = nc.dram_tensor("w3t", [128, 2, 256], BF16, kind="ExternalInput").ap()
    scd = nc.dram_tensor("scal", [128, 6], F32, kind="ExternalInput").ap()
    oned = nc.dram_tensor("onesrow", [1, HW], BF16, kind="ExternalInput").ap()
    idd = nc.dram_tensor("ident", [128, 128], BF16, kind="ExternalInput").ap()
    outd = nc.dram_tensor("out", [S_PER_CORE, 128, 2, HW], F32, kind="ExternalOutput").ap()

    with TileContext(nc) as tc, ExitStack() as ctx:
        consts = ctx.enter_context(tc.tile_pool(name="consts", bufs=1))
        xbpool = ctx.enter_context(tc.tile_pool(name="xb", bufs=2))
        ypapool = ctx.enter_context(tc.tile_pool(name="ypa", bufs=2))
        ypbpool = ctx.enter_context(tc.tile_pool(name="ypb", bufs=2))
        rpool = ctx.enter_context(tc.tile_pool(name="rp", bufs=2))
        wdpool = ctx.enter_context(tc.tile_pool(name="wdp", bufs=6))
        prpool = ctx.enter_context(tc.tile_pool(name="prp", bufs=9))
        cspool = ctx.enter_context(tc.tile_pool(name="csp", bufs=2))
        y2pool = ctx.enter_context(tc.tile_pool(name="y2p", bufs=2))
        opool = ctx.enter_context(tc.tile_pool(name="op", bufs=4))
        pspool = ctx.enter_context(tc.tile_pool(name="psp", bufs=1, space="PSUM"))

        w1t = consts.tile([128, 2, 256], BF16)
        nc.sync.dma_start(out=w1t, in_=w1d)
        rwt = consts.tile([128, 2, 64], BF16)
        nc.sync.dma_start(out=rwt, in_=rwd)
        scred = consts.tile([64, 1], F32)
        nc.sync.dma_start(out=scred, in_=srd)
        srep = consts.tile([KR, 2 * 49 * 128], BF16)
        nc.sync.dma_start(out=srep, in_=spd)
        w3t = consts.tile([128, 2, 256], BF16)
        nc.sync.dma_start(out=w3t, in_=w3d)
        scal = consts.tile([128, 6], F32)
        nc.sync.dma_start(out=scal, in_=scd)
        ident = consts.tile([128, 128], BF16)
        nc.sync.dma_start(out=ident, in_=idd)

        srv = srep[:].rearrange("p (m t c) -> p m t c", m=2, t=49)
        scv = scal[:].rearrange("p (m k) -> p m k", m=2)

        # 448-col tile pairs packed into [128,1024] psum tiles (512-stripes)
        pairs = [(0, 2), (2, 2), (4, 2), (6, 1)]

        ST = [dict() for _ in range(S_PER_CORE)]

        def emit_load(s):
            st = ST[s]
            xbt = xbpool.tile([128, 2, HW], BF16, tag="xb", name=f"xb{s}")
            nc.sync.dma_start(out=xbt, in_=xbd[s])
            st["xb"] = xbt

        def emit_conv1_start(s):
            st = ST[s]
            yat = ypapool.tile([128, 2, NPAD], BF16, tag="ypa", name=f"ya{s}")
            ybt = ypbpool.tile([128, 2, NPAD], BF16, tag="ypb", name=f"yb{s}")
            st["ypa"], st["ypb"] = yat, ybt
            # zero only the pad borders: top rows, bottom rows, inter-row strips
            yv = yat[:]
            nc.vector.memset(yv[:, :, 0:PAD * PW + PAD], 0.0)
            nc.vector.memset(yv[:, :, NPAD - PAD * PW - (PW - PAD - W):NPAD], 0.0)
            strip0 = PAD * PW + PAD + W
            strips = yv[:, :, strip0:strip0 + W * PW] \
                .rearrange("p m (h w) -> p m h w", w=PW)[:, :, :, 0:PW - W]
            nc.vector.memset(strips, 0.0)

        def emit_conv1_pair(s, m, t0, cnt):
            st = ST[s]
            yav = st["ypa"][:].rearrange("p m (h w) -> p m h w", w=PW)
            xbt = st["xb"]
            ps = pspool.tile([128, 1024], F32, tag="pss", bufs=3, name=f"c1p{s}{m}{t0}")
            psv = ps[:].rearrange("p (t w) -> p t w", w=512)
            for j in range(cnt):
                t = t0 + j
                for k in range(2):
                    nc.tensor.matmul(
                        psv[:, j, 0:T448],
                        w1t[:, k, m * 128:(m + 1) * 128],
                        xbt[:, k, t * T448:(t + 1) * T448],
                        start=(k == 0), stop=(k == 1),
                    )
            for j in range(cnt):
                t = t0 + j
                nc.scalar.activation(
                    out=yav[:, m, PAD + 8 * t:PAD + 8 * (t + 1), PAD:PAD + W],
                    in_=psv[:, j, 0:T448].rearrange("p (r w) -> p r w", w=W),
                    func=AF.Tanh,
                    scale=1.0,
                    bias=scv[:, m, 0:1],
                )

        def emit_conv1_end(s):
            st = ST[s]
            nc.vector.tensor_copy(
                st["ypb"][:, :, 0:NPAD - 1],
                st["ypa"][:, :, 1:NPAD],
            )

        def emit_red_start(s):
            st = ST[s]
            rt = rpool.tile([KR, HW], BF16, tag="r", name=f"r{s}")
            st["rt"] = rt
            nc.sync.dma_start(out=rt[KR - 1:KR, :], in_=oned)

        def emit_red_pair(s, t0, cnt):
            st = ST[s]
            yav = st["ypa"][:].rearrange("p m (h w) -> p m h w", w=PW)
            ps = pspool.tile([128, 1024], F32, tag="pss", bufs=3, name=f"rp{s}{t0}")
            psv = ps[:].rearrange("p (t w) -> p t w", w=512)
            for j in range(cnt):
                t = t0 + j
                for k in range(2):
                    rhs = yav[:, k, PAD + 8 * t:PAD + 8 * (t + 1), PAD:PAD + W]
                    nc.tensor.matmul(
                        psv[0:64, j, 0:T448],
                        rwt[:, k, :],
                        rhs,
                        start=(k == 0), stop=(k == 1),
                    )
            for j in range(cnt):
                t = t0 + j
                nc.scalar.activation(
                    out=st["rt"][0:64, t * T448:(t + 1) * T448],
                    in_=psv[0:64, j, 0:T448],
                    func=AF.Relu,
                    scale=1.0,
                    bias=scred[:],
                )

        def emit_inv_chunk(s, cix, fillers):
            """49-tap involution for one pixel-chunk (both m); adds trail the
            spans by two 3-tap batches; pops filler closures between batches."""
            st = ST[s]
            if "y2" not in st:
                st["y2"] = y2pool.tile([128, 2, HW], BF16, tag="y2", name=f"y2{s}")
            c0 = cix * CH
            r0 = cix * CROWS
            yav = st["ypa"][:].rearrange("p m (h w) -> p m h w", w=PW)
            ybv = st["ypb"][:].rearrange("p m (h w) -> p m h w", w=PW)
            rt = st["rt"]
            acc = pspool.tile([128, 1024], F32, tag="acc", bufs=1, name=f"acc{s}{cix}")
            chain = cspool.tile([128, 2 * CH], BF16, tag="cs", bufs=2,
                                name=f"cs{s}{cix}")
            pend = {}
            pe_first = [True]

            def flush_add(p):
                pr = pend.pop(p)
                if p in CHAIN_TAPS:
                    if p == CHAIN_TAPS[0]:
                        return  # pr IS the chain start
                    eng = nc.gpsimd if p in CHAIN_POOL else nc.vector
                    eng.tensor_tensor(chain[:], chain[:], pr[:], op=ALU.add)
                    return
                prv = pr[:].rearrange("p (m n) -> p m n", m=2)
                first = pe_first[0]
                pe_first[0] = False
                for m in range(2):
                    nc.tensor.matmul(
                        acc[:, m * 512:m * 512 + CH], ident[:], prv[:, m, :],
                        start=first, stop=False,
                    )

            def emit_tap(p):
                di, dj = p // KK, p % KK
                ps = pspool.tile([128, 1024], F32, tag="pss", bufs=3,
                                 name=f"sp{s}{cix}{p}")
                for m in range(2):
                    nc.tensor.matmul(
                        ps[:, m * 512:m * 512 + CH],
                        srv[:, m, p],
                        rt[:, c0:c0 + CH],
                        start=True, stop=True,
                    )
                if dj % 2 == 0:
                    ybase, djo = yav, dj
                else:
                    ybase, djo = ybv, dj - 1
                ysh = ybase[:, :, r0 + di:r0 + di + CROWS, djo:djo + W]
                psw = ps[:].rearrange("p (m n) -> p m n", m=2)[:, :, 0:CH] \
                    .rearrange("p m (r w) -> p m r w", w=W)
                if p == CHAIN_TAPS[0]:
                    pr = chain
                else:
                    pr = prpool.tile([128, 2 * CH], BF16, tag="pr",
                                     name=f"pr{s}{cix}{p}")
                prw = pr[:].rearrange("p (m r w) -> p m r w", m=2, w=W)
                tt = TAP_TYPES[p]
                if tt == 'S':
                    nc.vector.tensor_mul(prw, psw, ysh)
                else:
                    wd = wdpool.tile([128, 2 * CH], BF16, tag="wd",
                                     name=f"wd{s}{cix}{p}")
                    nc.scalar.activation(
                        out=wd[:].rearrange("p (m n) -> p m n", m=2),
                        in_=ps[:].rearrange("p (m n) -> p m n", m=2)[:, :, 0:CH],
                        func=AF.Identity,
                        scale=1.0,
                        bias=0.0,
                    )
                    wdw = wd[:].rearrange("p (m r w) -> p m r w", m=2, w=W)
                    if tt == 'A':
                        # per-m 3D ops: 4D views run at 1x on hw
                        for m in range(2):
                            nc.vector.tensor_mul(
                                prw[:, m], wdw[:, m], ysh[:, m])
                    else:
                        nc.gpsimd.tensor_tensor(prw, wdw, ysh, op=ALU.mult)
                pend[p] = pr

            done = 0
            for p in range(55):
                lim = min(max(p - 6, 0), 49)
                while done < lim:
                    flush_add(done)
                    done += 1
                if p < 49:
                    emit_tap(p)
                if p in (12, 24, 36, 48) and fillers:
                    fillers.pop(0)()
            while done < 49:
                flush_add(done)
                done += 1
            # fold the SBUF chain into acc and close the accumulation group
            chv = chain[:].rearrange("p (m n) -> p m n", m=2)
            for m in range(2):
                nc.tensor.matmul(
                    acc[:, m * 512:m * 512 + CH], ident[:], chv[:, m, :],
                    start=False, stop=True,
                )
            for m in range(2):
                nc.scalar.activation(
                    out=st["y2"][:, m, c0:c0 + CH],
                    in_=acc[:, m * 512:m * 512 + CH],
                    func=AF.Tanh,
                    scale=1.0,
                    bias=scv[:, m, 1:2],
                )

        def emit_conv3_pair(s, m, t0, cnt):
            st = ST[s]
            xbt = st["xb"]
            y2 = st["y2"]
            ps = pspool.tile([128, 1024], F32, tag="pss", bufs=3, name=f"c3p{s}{m}{t0}")
            psv = ps[:].rearrange("p (t w) -> p t w", w=512)
            for j in range(cnt):
                t = t0 + j
                nc.tensor.matmul(
                    psv[:, j, 0:T448],
                    ident[:],
                    xbt[:, m, t * T448:(t + 1) * T448],
                    start=True, stop=False,
                )
                for k in range(2):
                    nc.tensor.matmul(
                        psv[:, j, 0:T448],
                        w3t[:, k, m * 128:(m + 1) * 128],
                        y2[:, k, t * T448:(t + 1) * T448],
                        start=False, stop=(k == 1),
                    )
            ot = opool.tile([128, 2 * T448], F32, tag="o", name=f"o{s}{m}{t0}")
            otv = ot[:].rearrange("p (t w) -> p t w", t=2)
            for j in range(cnt):
                nc.scalar.activation(
                    out=otv[:, j, :],
                    in_=psv[:, j, 0:T448],
                    func=AF.Identity,
                    scale=1.0,
                    bias=scv[:, m, 2:3],
                )
            nc.sync.dma_start(
                out=outd[s][:, m, t0 * T448:(t0 + cnt) * T448],
                in_=ot[:, 0:cnt * T448],
            )

        def conv1_units(s):
            units = [lambda s=s: emit_conv1_start(s)]
            for m in range(2):
                for t0, cnt in pairs:
                    units.append(lambda s=s, m=m, t0=t0, cnt=cnt: emit_conv1_pair(s, m, t0, cnt))
            units.append(lambda s=s: emit_conv1_end(s))
            return units

        def red_units(s):
            units = [lambda s=s: emit_red_start(s)]
            for t0, cnt in pairs:
                units.append(lambda s=s, t0=t0, cnt=cnt: emit_red_pair(s, t0, cnt))
            return units

        def conv3_units(s):
            units = []
            for m in range(2):
                for t0, cnt in pairs:
                    units.append(lambda s=s, m=m, t0=t0, cnt=cnt: emit_conv3_pair(s, m, t0, cnt))
            return units

        # ---- schedule: prologue, then involutions with pipelined fillers ----
        emit_load(0)
        for u in conv1_units(0):
            u()
        if S_PER_CORE > 1:
            emit_load(1)
        for u in red_units(0):
            u()

        fillers0 = conv1_units(1) + red_units(1) if S_PER_CORE > 1 else []
        for cix in range(NCH):
            emit_inv_chunk(0, cix, fillers0)
        for u in fillers0:
            u()

        fillers1 = conv3_units(0)
        if S_PER_CORE > 1:
            for cix in range(NCH):
                emit_inv_chunk(1, cix, fillers1)
        for u in fillers1:
            u()
        if S_PER_CORE > 1:
            for u in conv3_units(1):
                u()

    nc.compile()
    return nc


def _bn_fold(g, b, m, v):
    s = (g / np.sqrt(v + EPS)).astype(np.float32)
    return s, (b - m * s).astype(np.float32)


def _prep_inputs(inputs):
    bf = ml_dtypes.bfloat16
    f32 = np.float32

    s1, t1 = _bn_fold(inputs["bn1_g"], inputs["bn1_b"], inputs["bn1_m"], inputs["bn1_v"])
    bias1 = (t1 + s1 * inputs["b1"]).astype(f32)
    w1f = (inputs["w1"] * s1[:, None]).astype(f32)
    sr, tr = _bn_fold(inputs["red_bn_g"], inputs["red_bn_b"], inputs["red_bn_m"], inputs["red_bn_v"])
    biasr = (tr + sr * inputs["red_b"]).astype(f32)
    rwf = (inputs["red_w"] * sr[:, None]).astype(f32)
    s2, t2 = _bn_fold(inputs["bn2_g"], inputs["bn2_b"], inputs["bn2_m"], inputs["bn2_v"])
    s3, t3 = _bn_fold(inputs["bn3_g"], inputs["bn3_b"], inputs["bn3_m"], inputs["bn3_v"])
    bias3 = (t3 + s3 * inputs["b3"]).astype(f32)
    w3f = (inputs["w3"] * s3[:, None]).astype(f32)

    w1t = np.ascontiguousarray(w1f.T.reshape(2, 128, 256).transpose(1, 0, 2)).astype(bf)
    rwt = np.ascontiguousarray(rwf.T.reshape(2, 128, 64).transpose(1, 0, 2)).astype(bf)
    w3t = np.ascontiguousarray(w3f.T.reshape(2, 128, 256).transpose(1, 0, 2)).astype(bf)
    scred = biasr.reshape(64, 1).astype(f32)

    # srep [65, 2m, 49p, 128c']: s2[c]*span_w[(g, p), e] with the ones-row
    # (q=64) carrying s2[c]*span_b[(g, p)]
    sw = inputs["span_w"].reshape(G, 49, RED)      # [g, p, e]
    sb = inputs["span_b"].reshape(G, 49)           # [g, p]
    srep = np.zeros((KR, 2, 49, 128), f32)
    cpr = np.arange(128)
    for m in range(2):
        gidx = (cpr // GC) + 8 * m                 # [c'] group index
        s2m = s2[cpr + 128 * m]                    # [c']
        swg = sw[gidx]                             # [c', p, e]
        srep[0:64, m] = (swg * s2m[:, None, None]).transpose(2, 1, 0)
        srep[64, m] = (sb[gidx] * s2m[:, None]).T
    srep = np.ascontiguousarray(srep.reshape(KR, -1)).astype(bf)

    scal = np.stack([bias1, t2, bias3], axis=-1)   # [256, 3]
    scal = np.ascontiguousarray(scal.reshape(2, 128, 3).transpose(1, 0, 2).reshape(128, 6)).astype(f32)

    onesrow = np.ones((1, HW), f32).astype(bf)
    ident = np.eye(128, dtype=f32).astype(bf)

    x = inputs["x"].reshape(B, 2, 128, HW).transpose(0, 2, 1, 3)

    common = dict(w1t=w1t, rwt=rwt, scred=scred, srep=srep, w3t=w3t, scal=scal,
                  onesrow=onesrow, ident=ident)
    in_maps = []
    for i in range(N_CORES):
        shard = np.ascontiguousarray(x[i * S_PER_CORE:(i + 1) * S_PER_CORE])
        in_maps.append({**common, "xb": shard.astype(bf)})
    return in_maps


_NC = None


def _get_nc():
    global _NC
    if _NC is None:
        _NC = _build_nc()
    return _NC


def kernel(**inputs):
    inputs = {k: np.asarray(v) for k, v in inputs.items()}
    nc = _get_nc()
    in_maps = _prep_inputs(inputs)
    res = run_bass_kernel_spmd(nc, in_maps, list(range(N_CORES)))
    outs = [
        res.results[i]["out"].transpose(0, 2, 1, 3).reshape(S_PER_CORE, C, H, W)
        for i in range(N_CORES)
    ]
    return np.concatenate(outs, axis=0).astype(np.float32)
